# revision 1
# baseline (speedup 1.0000x reference)
import numpy as np
import jax
import jax.numpy as jnp

# nn_AttentionSequencePoolingLayer: hardcoded problem shapes
B, T, E = 4096, 200, 64
NDEV = 8
BL = B // NDEV  # 512 batches per core


def _forward(queries, keys, keys_length, W1, b1, W2, b2, W3, b3):
    # LocalActivationUnit: concat [q, k, q-k, q*k] -> sigmoid MLP -> score
    q = jnp.broadcast_to(queries, keys.shape)                    # [b,T,E]
    att_in = jnp.concatenate([q, keys, q - keys, q * keys], -1)  # [b,T,4E]
    h = jax.nn.sigmoid(att_in @ W1 + b1)                         # [b,T,H1]
    h = jax.nn.sigmoid(h @ W2 + b2)                              # [b,T,H2]
    score = h @ W3 + b3                                          # [b,T,1]
    logits = jnp.swapaxes(score, 1, 2)                           # [b,1,T]
    key_mask = jnp.arange(T)[None, None, :] < keys_length[:, None, None]
    NEG = jnp.float32(-(2.0 ** 32) + 1.0)
    logits = jnp.where(key_mask, logits, NEG)
    weights = jax.nn.softmax(logits, axis=-1)                    # [b,1,T]
    return jnp.matmul(weights, keys)                             # [b,1,E]


_pfwd = jax.pmap(
    _forward,
    in_axes=(0, 0, 0, None, None, None, None, None, None),
)


def kernel(queries, keys, keys_length, W1, b1, W2, b2, W3, b3):
    if len(jax.devices()) >= NDEV:
        qs = np.ascontiguousarray(queries.reshape(NDEV, BL, 1, E))
        ks = np.ascontiguousarray(keys.reshape(NDEV, BL, T, E))
        kl = np.ascontiguousarray(keys_length.reshape(NDEV, BL))
        out = _pfwd(qs, ks, kl, W1, b1, W2, b2, W3, b3)
    else:
        out = jax.jit(_forward)(queries, keys, keys_length, W1, b1, W2, b2, W3, b3)
    return np.asarray(out).reshape(B, 1, E).astype(np.float32)

